# revision 19
# baseline (speedup 1.0000x reference)
"""Multi-head attention (B=2, S=2048, D=1024, H=16, RoPE) on 8 Trainium2 cores.

Sharding: tensor-parallel over heads. Core c owns heads (2c, 2c+1):
 - W_qkv column-sliced to that head pair (q|k|v blocks of 128 cols each),
 - W_out row-sliced to the pair's 128 input dims,
 - every core reads all tokens (x shipped pre-transposed as x^T, bf16),
 - each core emits a partial [4096, 1024] fp32 output; host sums the 8
   partials and adds b_out (the Megatron-style allreduce done on host).

Device program (per core, identical SPMD; all matmul operands bf16, fp32
PSUM accumulation):
  Projection in 512-token chunks: qkv^T = W_c^T @ x^T (8 K-block
  accumulation per 128-dim group); the qkv bias is fused into the PSUM
  evacuation as a per-partition tensor_scalar add. RoPE =
  (P2^T q^T) * sin + q^T * cos with P2 the rotate-half +/-1 permutation
  built on device. V is PE-transposed (bf16) into [t, d] layout.
  K^T is a single [128, S] tile per batch: head A dims on partitions
  0:64, head B on 64:128.
  Attention per (batch, 512-query chunk), software-pipelined over 128-key
  blocks: the two heads' K=64 score matmuls run CONCURRENTLY as PE
  row-tiles (tile_position (0,0) / (64,0)) into one 2-bank PSUM tile;
  one batched exp on ACT (1/8 scale folded in); attn@V accumulated in
  PSUM. The V layout [V_A | 1 | zeros | 1 | V_B] makes head A land on
  PSUM rows 0-64 and head B on rows 64-128 with softmax denominators in
  rows 64/63 for free, so the divided attention output assembles into
  one dense [128, n] operand and the output projection is a single
  K=128 matmul per tile. 1/l: DVE reciprocal directly on the [1, 512]
  denominator rows, then GpSimd partition_broadcast (no DRAM bounce).
  Batch 1's projections interleave with batch 0's attention.
"""

import os
import sys

if "/opt/trn_rl_repo" not in sys.path:
    sys.path.insert(0, "/opt/trn_rl_repo")

import numpy as np
import ml_dtypes

import concourse.bacc as bacc
import concourse.mybir as mybir
from concourse import masks
from concourse.tile import TileContext
from concourse.bass_utils import run_bass_kernel_spmd

F32 = mybir.dt.float32
BF16 = mybir.dt.bfloat16
ADD = mybir.AluOpType.add
MUL = mybir.AluOpType.mult
EXP = mybir.ActivationFunctionType.Exp

B, S, D, H, DH = 2, 2048, 1024, 16, 64
S2 = B * S              # 4096 tokens total
CH = 512                # token chunk for the projection phase
CPB = S // CH           # 4 chunks per batch
NSC = 4                 # 512-query chunks per batch
NTB = S // 128          # 16 key blocks per batch
VG = 256                # V2 group: VA(64) | 1 | zeros(63) | 1 | VB(64) | zeros(63)
                        # (both AV matmuls padded to M=128 bf16 weight blocks)


def _build_program():
    nc = bacc.Bacc("TRN2", target_bir_lowering=False, debug=False, num_devices=8)

    xT = nc.dram_tensor("xT", [D, S2], BF16, kind="ExternalInput")
    W = nc.dram_tensor("W", [D, 384], BF16, kind="ExternalInput")
    bqc = nc.dram_tensor("bqc", [128, 3], F32, kind="ExternalInput")
    Wo = nc.dram_tensor("Wo", [128, 1024], BF16, kind="ExternalInput")
    ctab_d = nc.dram_tensor("ctab", [128, S], F32, kind="ExternalInput")
    stab_d = nc.dram_tensor("stab", [128, S], F32, kind="ExternalInput")
    out_d = nc.dram_tensor("out", [S2, D], F32, kind="ExternalOutput")

    xT_re = xT.rearrange("(kb p) n -> p kb n", p=128)   # [128, 8, 4096]
    W_re = W.rearrange("(kb p) m -> p kb m", p=128)     # [128, 8, 384]

    with TileContext(nc) as tc:
        with tc.tile_pool(name="consts", bufs=1) as cp, \
             tc.tile_pool(name="xr", bufs=3) as xrp, \
             tc.tile_pool(name="pre", bufs=4) as prep, \
             tc.tile_pool(name="tmp", bufs=4) as tmpp, \
             tc.tile_pool(name="pt", bufs=4) as ptp, \
             tc.tile_pool(name="mrgs", bufs=3) as mrgs, \
             tc.tile_pool(name="lt", bufs=2) as ltp, \
             tc.tile_pool(name="a2c", bufs=6) as a2cp, \
             tc.tile_pool(name="osb", bufs=3) as osbp, \
             tc.tile_pool(name="dram", bufs=4, space="DRAM") as drp, \
             tc.tile_pool(name="pssc", bufs=2, space="PSUM") as pssc, \
             tc.tile_pool(name="psacc", bufs=1, space="PSUM") as psacc, \
             tc.tile_pool(name="small", bufs=2, space="PSUM") as smallp:

            W_r = cp.tile([128, 8 * 384], BF16, tag="W_r")
            Wo_r = cp.tile([128, 1024], BF16, tag="Wo_r")
            bq_col = cp.tile([128, 3], F32, tag="bq_col")
            ctab = cp.tile([128, S], F32, tag="ctab")
            stab = cp.tile([128, S], F32, tag="stab")
            ident = cp.tile([128, 128], BF16, tag="ident")
            P2r = cp.tile([128, 128], BF16, tag="P2r")
            qTb = [cp.tile([128, S], BF16, name=f"qT{b}", tag=f"qT{b}")
                   for b in range(B)]
            kTb = [cp.tile([128, S], BF16, name=f"kT{b}", tag=f"kT{b}")
                   for b in range(B)]
            V2b = [cp.tile([128, NTB * VG], BF16, name=f"V2{b}", tag=f"V2{b}")
                   for b in range(B)]

            # weights + bias first (first matmul dep); rope-table slices
            # are DMA'd per column chunk inside the batch-0 proj loop
            W_r_re = W_r[:].rearrange("p (kb m) -> p kb m", kb=8)
            nc.sync.dma_start(out=W_r_re[:, 0:2], in_=W_re[:, 0:2])
            nc.sync.dma_start(out=bq_col[:], in_=bqc[:])
            nc.sync.dma_start(out=W_r_re[:, 2:8], in_=W_re[:, 2:8])

            wrm = cp.tile([128, 512], BF16, tag="wrm")
            nc.gpsimd.memset(wrm[:], 0.25)

            def pe_warm(n, name):
                ps = psacc.tile([128, 1024], F32, tag="acc", name=f"warm{name}")
                for i in range(n):
                    nc.tensor.matmul(ps[:, 0:512], wrm[:, 0:128], wrm[:],
                                     start=True, stop=True)

            pe_warm(30, "start")

            identf = tmpp.tile([128, 128], F32, tag="tmp", name="identf")
            masks.make_identity(nc, identf[:])
            nc.vector.tensor_copy(ident[:], identf[:])

            # rotate-half matrix: P2[k, k^32] = -1 if (k%64)>=32 else +1
            p2f = tmpp.tile([128, 128], F32, tag="tmp", name="p2f")
            nc.gpsimd.memset(p2f[:], 0.0)
            for bk in (0, 64):
                nc.gpsimd.affine_select(
                    out=p2f[bk:bk + 32, :], in_=p2f[bk:bk + 32, :],
                    compare_op=mybir.AluOpType.not_equal, fill=1.0,
                    base=bk + 32, channel_multiplier=1, pattern=[[-1, 128]])
                nc.gpsimd.affine_select(
                    out=p2f[bk + 32:bk + 64, :], in_=p2f[bk + 32:bk + 64, :],
                    compare_op=mybir.AluOpType.not_equal, fill=-1.0,
                    base=bk, channel_multiplier=1, pattern=[[-1, 128]])
            nc.vector.tensor_copy(P2r[:], p2f[:])

            # V2: disjoint writers only -- ones columns, zero pad region;
            # VA/VB cols come from the chunk copies. No overlapping writes.
            for b in range(B):
                v2o = V2b[b][:].rearrange("p (g c) -> p g c", g=NTB)
                nc.gpsimd.memset(v2o[:, :, 64:65], 1.0)
                nc.gpsimd.memset(v2o[:, :, 65:191], 0.0)
                nc.gpsimd.memset(v2o[:, :, 191:192], 1.0)

            # ---------------- emitters ----------------------------------
            def chunk_stages(ch):
                # Issue the x DMA now (prefetch); return compute closures to
                # be interleaved with attention so the PE never serializes a
                # whole chunk between scs.
                bb, cb = ch // CPB, ch % CPB
                scol = cb * CH
                xr = xrp.tile([128, 8 * CH], BF16, tag="xr", name=f"xr{ch}")
                xr_re = xr[:].rearrange("p (kb n) -> p kb n", kb=8)
                nc.sync.dma_start(
                    out=xr_re[:, 0:2], in_=xT_re[:, 0:2, ch * CH:(ch + 1) * CH])
                nc.sync.dma_start(
                    out=xr_re[:, 2:8], in_=xT_re[:, 2:8, ch * CH:(ch + 1) * CH])
                pres = {}
                csl = slice(scol, scol + CH)

                psd = {}

                def qkv_piece(mt, piece):
                    # 2 of the 8 K-block matmuls: fine-grained pops keep each
                    # PE burst under the ~0.4us/tb ACT slack
                    if piece == 0:
                        psd[mt] = smallp.tile([128, CH], F32, tag="small",
                                              name=f"qkv{ch}_{mt}")
                    ps = psd[mt]
                    for kb in (2 * piece, 2 * piece + 1):
                        nc.tensor.matmul(
                            ps[:],
                            W_r[:, kb * 384 + mt * 128:kb * 384 + (mt + 1) * 128],
                            xr[:, kb * CH:(kb + 1) * CH],
                            start=(kb == 0), stop=(kb == 7))
                    if piece == 3:
                        pre = prep.tile([128, CH], BF16, tag="pre",
                                        name=f"pre{ch}_{mt}")
                        nc.vector.tensor_scalar_add(
                            out=pre[:], in0=ps[:], scalar1=bq_col[:, mt:mt + 1])
                        pres[mt] = pre

                def rope_stage(mt):
                    pre = pres[mt]
                    rot = smallp.tile([128, CH], F32, tag="small",
                                      name=f"rot{ch}_{mt}")
                    nc.tensor.matmul(rot[:], P2r[:], pre[:], start=True, stop=True)
                    t1 = tmpp.tile([128, CH], F32, tag="tmp", name=f"t1_{ch}_{mt}")
                    nc.vector.tensor_tensor(
                        out=t1[:], in0=rot[:], in1=stab[:, csl], op=MUL)
                    t2 = tmpp.tile([128, CH], F32, tag="tmp", name=f"t2_{ch}_{mt}")
                    nc.vector.tensor_tensor(
                        out=t2[:], in0=pre[:], in1=ctab[:, csl], op=MUL)
                    dstT = qTb[bb] if mt == 0 else kTb[bb]
                    nc.vector.tensor_tensor(
                        out=dstT[:, csl], in0=t1[:], in1=t2[:], op=ADD)

                vsd = {}

                def v_piece(piece):
                    if piece == 0:
                        vsd[0] = smallp.tile([128, CH], BF16, tag="small",
                                             name=f"v2p{ch}")
                    v2p = vsd[0]
                    for i in (2 * piece, 2 * piece + 1):
                        nc.tensor.transpose(
                            v2p[:, i * 128:(i + 1) * 128],
                            pres[2][:, i * 128:(i + 1) * 128], ident[:])
                    if piece == 1:
                        g0 = cb * 4
                        dst = V2b[bb][:, g0 * VG:(g0 + 4) * VG].rearrange(
                            "p (i c) -> p i c", i=4)
                        src = v2p[:].rearrange("p (i h d) -> p i h d", i=4, h=2)
                        nc.vector.tensor_copy(dst[:, :, 0:64], src[:, :, 0:1, :])
                        nc.vector.tensor_copy(dst[:, :, 192:256],
                                              src[:, :, 1:2, :])

                return ([lambda mt=mt, p=p: qkv_piece(mt, p)
                         for mt in range(3) for p in range(4)]
                        + [lambda: rope_stage(0), lambda: rope_stage(1),
                           lambda: v_piece(0), lambda: v_piece(1)])

            pending_proj = []

            def emit_chunk(ch):
                for fn in chunk_stages(ch):
                    fn()

            pending_out = []
            pending_tail = []

            def emit_sc(bb, sc, pop_oms=True):
                qcol = sc * 512
                qT, kT, V2 = qTb[bb], kTb[bb], V2b[bb]
                gam = psacc.tile([128, 1024], F32, tag="acc",
                                 name=f"gam{bb}_{sc}")

                def av_mms(tb, pa):
                    gcol = tb * VG
                    nc.tensor.matmul(
                        gam[:, 0:512],
                        V2[:, gcol:gcol + 128], pa[:, 0:512],
                        start=(tb == 0), stop=(tb == NTB - 1))
                    nc.tensor.matmul(
                        gam[:, 512:1024],
                        V2[:, gcol + 128:gcol + 256], pa[:, 512:1024],
                        start=(tb == 0), stop=(tb == NTB - 1))

                lag = []
                for tb in range(NTB):
                    tcol = tb * 128
                    sco = pssc.tile([128, 1024], F32, tag="score",
                                    name=f"sco{bb}_{sc}_{tb}")
                    # both heads concurrently as K=64 PE row-tiles
                    nc.tensor.matmul(
                        sco[:, 0:512],
                        kT[0:64, tcol:tcol + 128], qT[0:64, qcol:qcol + 512],
                        start=True, stop=True)
                    nc.tensor.matmul(
                        sco[:, 512:1024],
                        kT[64:128, tcol:tcol + 128], qT[64:128, qcol:qcol + 512],
                        start=True, stop=True)
                    pa = ptp.tile([128, 1024], BF16, tag="pt",
                                  name=f"pa{bb}_{sc}_{tb}")
                    nc.scalar.activation(pa[:], sco[:], EXP, scale=0.125)
                    lag.append((tb, pa))
                    if tb == 0 and pending_tail:
                        pending_tail.pop(0)()
                    if len(lag) > 2:
                        av_mms(*lag.pop(0))
                    # interleave deferred work: projection stages of the
                    # other batch fill ACT-wait slack; output-projections only
                    # pop late so their a2 (prev sc's ~10us divide chain) is
                    # ready -- a blocked om matmul at the PE FIFO head stalls
                    # everything
                    if pending_proj:
                        pending_proj.pop(0)()
                    if pop_oms and pending_out and tb >= 8:
                        pending_out.pop(0)()
                def sc_tail():
                    for e in lag:
                        av_mms(*e)
                    finish_sc(bb, sc, qcol, gam)
                pending_tail.append(sc_tail)

            def finish_sc(bb, sc, qcol, gam):
                # merge + divide: head A rows 0:64 (l at row 64 of gam-A),
                # head B rows 64:128 (l at row 63 of gam-B). The l rows
                # bounce through DRAM to land on partition 0 for the
                # reciprocal + partition broadcast.
                smg = mrgs.tile([128, 512], F32, tag="smg", name=f"smg{bb}_{sc}")
                nc.vector.tensor_copy(smg[0:65, :], gam[0:65, 0:512])
                lsA = drp.tile([512], F32, tag="lscr", name=f"lsA{bb}_{sc}")
                nc.sync.dma_start(out=lsA[None, :], in_=smg[64:65, :])
                lb = mrgs.tile([128, 512], F32, tag="lb", name=f"lb{bb}_{sc}")
                nc.vector.tensor_copy(lb[32:64, :], gam[32:64, 512:1024])
                nc.vector.tensor_copy(smg[64:128, :], gam[64:128, 512:1024])
                lsB = drp.tile([512], F32, tag="lscr", name=f"lsB{bb}_{sc}")
                nc.sync.dma_start(out=lsB[None, :], in_=lb[63:64, :])
                a2 = a2cp.tile([128, 512], BF16, tag="a2c", name=f"a2c{bb}_{sc}")
                for h, lscr in ((0, lsA), (1, lsB)):
                    # broadcast l first, then full-width approx reciprocal
                    # (51 ULP): one DMA hop instead of three
                    rl1 = ltp.tile([128, 512], F32, tag="rl1",
                                   name=f"rl1_{bb}{sc}{h}")
                    nc.sync.dma_start(out=rl1[0:1, :], in_=lscr[None, :])
                    lful = ltp.tile([128, 512], F32, tag="lful",
                                    name=f"lf_{bb}{sc}{h}")
                    nc.gpsimd.partition_broadcast(out_ap=lful[:], in_ap=rl1[0:1, :])
                    rlb = ltp.tile([128, 512], F32, tag="rlb",
                                   name=f"rlb_{bb}{sc}{h}")
                    nc.vector.reciprocal_approx_fast(out=rlb[:], in_=lful[:])
                    nc.vector.tensor_tensor(
                        out=a2[64 * h:64 * (h + 1), :],
                        in0=smg[64 * h:64 * (h + 1), :],
                        in1=rlb[64 * h:64 * (h + 1), :], op=MUL)

                def make_outproj(bb, qcol, a2):
                    def emit_nb(nb):
                        o = osbp.tile([128, 1024], F32, tag="osb",
                                      name=f"osb{bb}_{qcol}_{nb}")
                        for jc in range(2):
                            om = smallp.tile([128, 512], F32, tag="small",
                                             name=f"om{bb}_{qcol}_{nb}_{jc}")
                            nc.tensor.matmul(
                                om[:], a2[:, nb * 128:(nb + 1) * 128],
                                Wo_r[:, jc * 512:(jc + 1) * 512],
                                start=True, stop=True)
                            nc.vector.tensor_copy(
                                o[:, jc * 512:(jc + 1) * 512], om[:])
                        nc.sync.dma_start(
                            out=out_d[bb * S + qcol + nb * 128:
                                      bb * S + qcol + (nb + 1) * 128, :],
                            in_=o[:])
                    return [lambda nb=nb: emit_nb(nb) for nb in range(4)]

                pending_out.extend(make_outproj(bb, qcol, a2))

            # ---------------- schedule ----------------------------------
            for ch in range(CPB):           # batch 0 projections
                stages = chunk_stages(ch)       # issues the x DMA first
                csl = slice(ch * CH, (ch + 1) * CH)
                nc.sync.dma_start(out=ctab[:, csl], in_=ctab_d[:, csl])
                nc.sync.dma_start(out=stab[:, csl], in_=stab_d[:, csl])
                for fn in stages:
                    fn()
            nc.sync.dma_start(out=Wo_r[:], in_=Wo[:])
            for sc in range(NSC):           # batch 0 attention || batch 1 proj
                pending_proj.extend(chunk_stages(CPB + sc))
                emit_sc(0, sc, pop_oms=False)
            for sc in range(NSC):           # batch 1 attention
                emit_sc(1, sc)
            for fn in pending_tail:
                fn()
            pe_warm(16, "tail")
            for fn in pending_out:
                fn()

    nc.compile()
    return nc


_PROG = None


def _get_program():
    global _PROG
    if _PROG is None:
        _PROG = _build_program()
    return _PROG


def _rope_tables():
    inv_freq = (1.0 / (10000.0 ** (np.arange(0, DH, 2, dtype=np.float32) / DH)))
    invf2 = inv_freq[np.arange(128) % 32]
    ang = np.arange(S, dtype=np.float32)[None, :] * invf2[:, None].astype(np.float32)
    return (np.cos(ang).astype(np.float32), np.sin(ang).astype(np.float32))


def make_in_maps(x, W_qkv, b_qkv, W_out, b_out):
    BF = ml_dtypes.bfloat16
    x = np.asarray(x, dtype=np.float32)
    W_qkv = np.asarray(W_qkv, dtype=np.float32)
    b_qkv = np.asarray(b_qkv, dtype=np.float32)
    W_out = np.asarray(W_out, dtype=np.float32)

    xT = np.ascontiguousarray(x.reshape(S2, D).T.astype(BF))
    ct, st = _rope_tables()

    in_maps = []
    for c in range(8):
        hA, hB = 2 * c, 2 * c + 1
        cols = np.r_[hA * DH:(hA + 1) * DH, hB * DH:(hB + 1) * DH]
        Wc = np.ascontiguousarray(
            np.concatenate([W_qkv[:, off + cols] for off in (0, D, 2 * D)],
                           axis=1).astype(BF))
        bqc = np.ascontiguousarray(
            np.concatenate([b_qkv[off + cols] for off in (0, D, 2 * D)])
            .reshape(3, 128).T.astype(np.float32))
        Woc = np.ascontiguousarray(W_out[c * 128:(c + 1) * 128, :].astype(BF))
        in_maps.append(
            {"xT": xT, "W": Wc, "bqc": bqc, "Wo": Woc, "ctab": ct, "stab": st})
    return in_maps


def assemble_output(results, b_out):
    acc = results[0]["out"].astype(np.float64)
    for c in range(1, 8):
        acc += results[c]["out"]
    out = acc + np.asarray(b_out, dtype=np.float64)
    return out.reshape(B, S, D).astype(np.float32)


def kernel(x, W_qkv, b_qkv, W_out, b_out):
    nc = _get_program()
    in_maps = make_in_maps(x, W_qkv, b_qkv, W_out, b_out)
    res = run_bass_kernel_spmd(nc, in_maps, core_ids=list(range(8)))
    return assemble_output(res.results, b_out)


if __name__ == "__main__":
    rng = np.random.default_rng(0)
    ins = {
        "x": rng.standard_normal((B, S, D), dtype=np.float32),
        "W_qkv": rng.standard_normal((D, 3 * D), dtype=np.float32) / 32.0,
        "b_qkv": np.zeros(3 * D, np.float32),
        "W_out": rng.standard_normal((D, D), dtype=np.float32) / 32.0,
        "b_out": np.zeros(D, np.float32),
    }
    o = kernel(**ins)
    print("kernel ran:", o.shape, o.dtype)


# revision 20
# speedup vs baseline: 1.0228x; 1.0228x over previous
"""Multi-head attention (B=2, S=2048, D=1024, H=16, RoPE) on 8 Trainium2 cores.

Sharding: tensor-parallel over heads. Core c owns heads (2c, 2c+1):
 - W_qkv column-sliced to that head pair (q|k|v blocks of 128 cols each),
 - W_out row-sliced to the pair's 128 input dims,
 - every core reads all tokens (x shipped pre-transposed as x^T, bf16),
 - each core emits a partial [4096, 1024] fp32 output; host sums the 8
   partials and adds b_out (the Megatron-style allreduce done on host).

Device program (per core, identical SPMD; all matmul operands bf16, fp32
PSUM accumulation):
  Projection in 512-token chunks: qkv^T = W_c^T @ x^T (8 K-block
  accumulation per 128-dim group); the qkv bias is fused into the PSUM
  evacuation as a per-partition tensor_scalar add. RoPE =
  (P2^T q^T) * sin + q^T * cos with P2 the rotate-half +/-1 permutation
  built on device. V is PE-transposed (bf16) into [t, d] layout.
  K^T is a single [128, S] tile per batch: head A dims on partitions
  0:64, head B on 64:128.
  Attention per (batch, 512-query chunk), software-pipelined over 128-key
  blocks: the two heads' K=64 score matmuls run CONCURRENTLY as PE
  row-tiles (tile_position (0,0) / (64,0)) into one 2-bank PSUM tile;
  one batched exp on ACT (1/8 scale folded in); attn@V accumulated in
  PSUM. The V layout [V_A | 1 | zeros | 1 | V_B] makes head A land on
  PSUM rows 0-64 and head B on rows 64-128 with softmax denominators in
  rows 64/63 for free, so the divided attention output assembles into
  one dense [128, n] operand and the output projection is a single
  K=128 matmul per tile. 1/l: DVE reciprocal directly on the [1, 512]
  denominator rows, then GpSimd partition_broadcast (no DRAM bounce).
  Batch 1's projections interleave with batch 0's attention.
"""

import os
import sys

if "/opt/trn_rl_repo" not in sys.path:
    sys.path.insert(0, "/opt/trn_rl_repo")

import numpy as np
import ml_dtypes

import concourse.bacc as bacc
import concourse.mybir as mybir
from concourse import masks
from concourse.tile import TileContext
from concourse.bass_utils import run_bass_kernel_spmd

F32 = mybir.dt.float32
BF16 = mybir.dt.bfloat16
ADD = mybir.AluOpType.add
MUL = mybir.AluOpType.mult
EXP = mybir.ActivationFunctionType.Exp

B, S, D, H, DH = 2, 2048, 1024, 16, 64
S2 = B * S              # 4096 tokens total
CH = 512                # token chunk for the projection phase
CPB = S // CH           # 4 chunks per batch
NSC = 4                 # 512-query chunks per batch
NTB = S // 128          # 16 key blocks per batch
VG = 256                # V2 group: VA(64) | 1 | zeros(63) | 1 | VB(64) | zeros(63)
                        # (both AV matmuls padded to M=128 bf16 weight blocks)


def _build_program():
    nc = bacc.Bacc("TRN2", target_bir_lowering=False, debug=False, num_devices=8)

    xT = nc.dram_tensor("xT", [D, S2], BF16, kind="ExternalInput")
    W = nc.dram_tensor("W", [D, 384], BF16, kind="ExternalInput")
    bqc = nc.dram_tensor("bqc", [128, 3], F32, kind="ExternalInput")
    Wo = nc.dram_tensor("Wo", [128, 1024], BF16, kind="ExternalInput")
    ctab_d = nc.dram_tensor("ctab", [128, S], F32, kind="ExternalInput")
    stab_d = nc.dram_tensor("stab", [128, S], F32, kind="ExternalInput")
    out_d = nc.dram_tensor("out", [S2, D], BF16, kind="ExternalOutput")

    xT_re = xT.rearrange("(kb p) n -> p kb n", p=128)   # [128, 8, 4096]
    W_re = W.rearrange("(kb p) m -> p kb m", p=128)     # [128, 8, 384]

    with TileContext(nc) as tc:
        with tc.tile_pool(name="consts", bufs=1) as cp, \
             tc.tile_pool(name="xr", bufs=3) as xrp, \
             tc.tile_pool(name="pre", bufs=4) as prep, \
             tc.tile_pool(name="tmp", bufs=4) as tmpp, \
             tc.tile_pool(name="pt", bufs=4) as ptp, \
             tc.tile_pool(name="mrgs", bufs=3) as mrgs, \
             tc.tile_pool(name="lt", bufs=2) as ltp, \
             tc.tile_pool(name="a2c", bufs=6) as a2cp, \
             tc.tile_pool(name="osb", bufs=3) as osbp, \
             tc.tile_pool(name="dram", bufs=4, space="DRAM") as drp, \
             tc.tile_pool(name="pssc", bufs=2, space="PSUM") as pssc, \
             tc.tile_pool(name="psacc", bufs=1, space="PSUM") as psacc, \
             tc.tile_pool(name="small", bufs=2, space="PSUM") as smallp:

            W_r = cp.tile([128, 8 * 384], BF16, tag="W_r")
            Wo_r = cp.tile([128, 1024], BF16, tag="Wo_r")
            bq_col = cp.tile([128, 3], F32, tag="bq_col")
            ctab = cp.tile([128, S], F32, tag="ctab")
            stab = cp.tile([128, S], F32, tag="stab")
            ident = cp.tile([128, 128], BF16, tag="ident")
            P2r = cp.tile([128, 128], BF16, tag="P2r")
            qTb = [cp.tile([128, S], BF16, name=f"qT{b}", tag=f"qT{b}")
                   for b in range(B)]
            kTb = [cp.tile([128, S], BF16, name=f"kT{b}", tag=f"kT{b}")
                   for b in range(B)]
            V2b = [cp.tile([128, NTB * VG], BF16, name=f"V2{b}", tag=f"V2{b}")
                   for b in range(B)]

            # weights + bias first (first matmul dep); rope-table slices
            # are DMA'd per column chunk inside the batch-0 proj loop
            W_r_re = W_r[:].rearrange("p (kb m) -> p kb m", kb=8)
            nc.sync.dma_start(out=W_r_re[:, 0:2], in_=W_re[:, 0:2])
            nc.sync.dma_start(out=bq_col[:], in_=bqc[:])
            nc.sync.dma_start(out=W_r_re[:, 2:8], in_=W_re[:, 2:8])

            wrm = cp.tile([128, 512], BF16, tag="wrm")
            nc.gpsimd.memset(wrm[:], 0.25)

            def pe_warm(n, name):
                ps = psacc.tile([128, 1024], F32, tag="acc", name=f"warm{name}")
                for i in range(n):
                    nc.tensor.matmul(ps[:, 0:512], wrm[:, 0:128], wrm[:],
                                     start=True, stop=True)

            pe_warm(15, "start")

            identf = tmpp.tile([128, 128], F32, tag="tmp", name="identf")
            masks.make_identity(nc, identf[:])
            nc.vector.tensor_copy(ident[:], identf[:])

            # rotate-half matrix: P2[k, k^32] = -1 if (k%64)>=32 else +1
            p2f = tmpp.tile([128, 128], F32, tag="tmp", name="p2f")
            nc.gpsimd.memset(p2f[:], 0.0)
            for bk in (0, 64):
                nc.gpsimd.affine_select(
                    out=p2f[bk:bk + 32, :], in_=p2f[bk:bk + 32, :],
                    compare_op=mybir.AluOpType.not_equal, fill=1.0,
                    base=bk + 32, channel_multiplier=1, pattern=[[-1, 128]])
                nc.gpsimd.affine_select(
                    out=p2f[bk + 32:bk + 64, :], in_=p2f[bk + 32:bk + 64, :],
                    compare_op=mybir.AluOpType.not_equal, fill=-1.0,
                    base=bk, channel_multiplier=1, pattern=[[-1, 128]])
            nc.vector.tensor_copy(P2r[:], p2f[:])

            # V2: disjoint writers only -- ones columns, zero pad region;
            # VA/VB cols come from the chunk copies. No overlapping writes.
            for b in range(B):
                v2o = V2b[b][:].rearrange("p (g c) -> p g c", g=NTB)
                nc.gpsimd.memset(v2o[:, :, 64:65], 1.0)
                nc.gpsimd.memset(v2o[:, :, 65:191], 0.0)
                nc.gpsimd.memset(v2o[:, :, 191:192], 1.0)

            # ---------------- emitters ----------------------------------
            def chunk_stages(ch):
                # Issue the x DMA now (prefetch); return compute closures to
                # be interleaved with attention so the PE never serializes a
                # whole chunk between scs.
                bb, cb = ch // CPB, ch % CPB
                scol = cb * CH
                xr = xrp.tile([128, 8 * CH], BF16, tag="xr", name=f"xr{ch}")
                xr_re = xr[:].rearrange("p (kb n) -> p kb n", kb=8)
                nc.sync.dma_start(
                    out=xr_re[:, 0:2], in_=xT_re[:, 0:2, ch * CH:(ch + 1) * CH])
                nc.sync.dma_start(
                    out=xr_re[:, 2:8], in_=xT_re[:, 2:8, ch * CH:(ch + 1) * CH])
                pres = {}
                csl = slice(scol, scol + CH)

                psd = {}

                def qkv_piece(mt, piece):
                    # 2 of the 8 K-block matmuls: fine-grained pops keep each
                    # PE burst under the ~0.4us/tb ACT slack
                    if piece == 0:
                        psd[mt] = smallp.tile([128, CH], F32, tag="small",
                                              name=f"qkv{ch}_{mt}")
                    ps = psd[mt]
                    for kb in (2 * piece, 2 * piece + 1):
                        nc.tensor.matmul(
                            ps[:],
                            W_r[:, kb * 384 + mt * 128:kb * 384 + (mt + 1) * 128],
                            xr[:, kb * CH:(kb + 1) * CH],
                            start=(kb == 0), stop=(kb == 7))
                    if piece == 3:
                        pre = prep.tile([128, CH], BF16, tag="pre",
                                        name=f"pre{ch}_{mt}")
                        nc.vector.tensor_scalar_add(
                            out=pre[:], in0=ps[:], scalar1=bq_col[:, mt:mt + 1])
                        pres[mt] = pre

                def rope_stage(mt):
                    pre = pres[mt]
                    rot = smallp.tile([128, CH], F32, tag="small",
                                      name=f"rot{ch}_{mt}")
                    nc.tensor.matmul(rot[:], P2r[:], pre[:], start=True, stop=True)
                    t1 = tmpp.tile([128, CH], F32, tag="tmp", name=f"t1_{ch}_{mt}")
                    nc.vector.tensor_tensor(
                        out=t1[:], in0=rot[:], in1=stab[:, csl], op=MUL)
                    t2 = tmpp.tile([128, CH], F32, tag="tmp", name=f"t2_{ch}_{mt}")
                    nc.vector.tensor_tensor(
                        out=t2[:], in0=pre[:], in1=ctab[:, csl], op=MUL)
                    dstT = qTb[bb] if mt == 0 else kTb[bb]
                    nc.vector.tensor_tensor(
                        out=dstT[:, csl], in0=t1[:], in1=t2[:], op=ADD)

                vsd = {}

                def v_piece(piece):
                    if piece == 0:
                        vsd[0] = smallp.tile([128, CH], BF16, tag="small",
                                             name=f"v2p{ch}")
                    v2p = vsd[0]
                    for i in (2 * piece, 2 * piece + 1):
                        nc.tensor.transpose(
                            v2p[:, i * 128:(i + 1) * 128],
                            pres[2][:, i * 128:(i + 1) * 128], ident[:])
                    if piece == 1:
                        g0 = cb * 4
                        dst = V2b[bb][:, g0 * VG:(g0 + 4) * VG].rearrange(
                            "p (i c) -> p i c", i=4)
                        src = v2p[:].rearrange("p (i h d) -> p i h d", i=4, h=2)
                        nc.vector.tensor_copy(dst[:, :, 0:64], src[:, :, 0:1, :])
                        nc.vector.tensor_copy(dst[:, :, 192:256],
                                              src[:, :, 1:2, :])

                return ([lambda mt=mt, p=p: qkv_piece(mt, p)
                         for mt in range(3) for p in range(4)]
                        + [lambda: rope_stage(0), lambda: rope_stage(1),
                           lambda: v_piece(0), lambda: v_piece(1)])

            pending_proj = []

            def emit_chunk(ch):
                for fn in chunk_stages(ch):
                    fn()

            pending_out = []
            pending_tail = []

            def emit_sc(bb, sc, pop_oms=True):
                qcol = sc * 512
                qT, kT, V2 = qTb[bb], kTb[bb], V2b[bb]
                gam = psacc.tile([128, 1024], F32, tag="acc",
                                 name=f"gam{bb}_{sc}")

                def av_mms(tb, pa):
                    gcol = tb * VG
                    nc.tensor.matmul(
                        gam[:, 0:512],
                        V2[:, gcol:gcol + 128], pa[:, 0:512],
                        start=(tb == 0), stop=(tb == NTB - 1))
                    nc.tensor.matmul(
                        gam[:, 512:1024],
                        V2[:, gcol + 128:gcol + 256], pa[:, 512:1024],
                        start=(tb == 0), stop=(tb == NTB - 1))

                lag = []
                for tb in range(NTB):
                    tcol = tb * 128
                    sco = pssc.tile([128, 1024], F32, tag="score",
                                    name=f"sco{bb}_{sc}_{tb}")
                    # both heads concurrently as K=64 PE row-tiles
                    nc.tensor.matmul(
                        sco[:, 0:512],
                        kT[0:64, tcol:tcol + 128], qT[0:64, qcol:qcol + 512],
                        start=True, stop=True)
                    nc.tensor.matmul(
                        sco[:, 512:1024],
                        kT[64:128, tcol:tcol + 128], qT[64:128, qcol:qcol + 512],
                        start=True, stop=True)
                    pa = ptp.tile([128, 1024], BF16, tag="pt",
                                  name=f"pa{bb}_{sc}_{tb}")
                    nc.scalar.activation(pa[:], sco[:], EXP, scale=0.125)
                    lag.append((tb, pa))
                    if tb == 0 and pending_tail:
                        pending_tail.pop(0)()
                    if len(lag) > 2:
                        av_mms(*lag.pop(0))
                    # interleave deferred work: projection stages of the
                    # other batch fill ACT-wait slack; output-projections only
                    # pop late so their a2 (prev sc's ~10us divide chain) is
                    # ready -- a blocked om matmul at the PE FIFO head stalls
                    # everything
                    if pending_proj:
                        pending_proj.pop(0)()
                    if pop_oms and pending_out and tb >= 8:
                        pending_out.pop(0)()
                def sc_tail():
                    for e in lag:
                        av_mms(*e)
                    finish_sc(bb, sc, qcol, gam)
                pending_tail.append(sc_tail)

            def finish_sc(bb, sc, qcol, gam):
                # merge + divide: head A rows 0:64 (l at row 64 of gam-A),
                # head B rows 64:128 (l at row 63 of gam-B). The l rows
                # bounce through DRAM to land on partition 0 for the
                # reciprocal + partition broadcast.
                smg = mrgs.tile([128, 512], F32, tag="smg", name=f"smg{bb}_{sc}")
                nc.vector.tensor_copy(smg[0:65, :], gam[0:65, 0:512])
                lsA = drp.tile([512], F32, tag="lscr", name=f"lsA{bb}_{sc}")
                nc.sync.dma_start(out=lsA[None, :], in_=smg[64:65, :])
                lb = mrgs.tile([128, 512], F32, tag="lb", name=f"lb{bb}_{sc}")
                nc.vector.tensor_copy(lb[32:64, :], gam[32:64, 512:1024])
                nc.vector.tensor_copy(smg[64:128, :], gam[64:128, 512:1024])
                lsB = drp.tile([512], F32, tag="lscr", name=f"lsB{bb}_{sc}")
                nc.sync.dma_start(out=lsB[None, :], in_=lb[63:64, :])
                a2 = a2cp.tile([128, 512], BF16, tag="a2c", name=f"a2c{bb}_{sc}")
                for h, lscr in ((0, lsA), (1, lsB)):
                    # broadcast l first, then full-width approx reciprocal
                    # (51 ULP): one DMA hop instead of three
                    rl1 = ltp.tile([128, 512], F32, tag="rl1",
                                   name=f"rl1_{bb}{sc}{h}")
                    nc.sync.dma_start(out=rl1[0:1, :], in_=lscr[None, :])
                    lful = ltp.tile([128, 512], F32, tag="lful",
                                    name=f"lf_{bb}{sc}{h}")
                    nc.gpsimd.partition_broadcast(out_ap=lful[:], in_ap=rl1[0:1, :])
                    rlb = ltp.tile([128, 512], F32, tag="rlb",
                                   name=f"rlb_{bb}{sc}{h}")
                    nc.vector.reciprocal_approx_fast(out=rlb[:], in_=lful[:])
                    nc.vector.tensor_tensor(
                        out=a2[64 * h:64 * (h + 1), :],
                        in0=smg[64 * h:64 * (h + 1), :],
                        in1=rlb[64 * h:64 * (h + 1), :], op=MUL)

                def make_outproj(bb, qcol, a2):
                    def emit_nb(nb):
                        o = osbp.tile([128, 1024], BF16, tag="osb",
                                      name=f"osb{bb}_{qcol}_{nb}")
                        for jc in range(2):
                            om = smallp.tile([128, 512], F32, tag="small",
                                             name=f"om{bb}_{qcol}_{nb}_{jc}")
                            nc.tensor.matmul(
                                om[:], a2[:, nb * 128:(nb + 1) * 128],
                                Wo_r[:, jc * 512:(jc + 1) * 512],
                                start=True, stop=True)
                            nc.vector.tensor_copy(
                                o[:, jc * 512:(jc + 1) * 512], om[:])
                        nc.sync.dma_start(
                            out=out_d[bb * S + qcol + nb * 128:
                                      bb * S + qcol + (nb + 1) * 128, :],
                            in_=o[:])
                    return [lambda nb=nb: emit_nb(nb) for nb in range(4)]

                pending_out.extend(make_outproj(bb, qcol, a2))

            # ---------------- schedule ----------------------------------
            for ch in range(CPB):           # batch 0 projections
                stages = chunk_stages(ch)       # issues the x DMA first
                csl = slice(ch * CH, (ch + 1) * CH)
                nc.sync.dma_start(out=ctab[:, csl], in_=ctab_d[:, csl])
                nc.sync.dma_start(out=stab[:, csl], in_=stab_d[:, csl])
                for fn in stages:
                    fn()
            nc.sync.dma_start(out=Wo_r[:], in_=Wo[:])
            for sc in range(NSC):           # batch 0 attention || batch 1 proj
                pending_proj.extend(chunk_stages(CPB + sc))
                emit_sc(0, sc, pop_oms=False)
            for sc in range(NSC):           # batch 1 attention
                emit_sc(1, sc)
            for fn in pending_tail:
                fn()
            pe_warm(16, "tail")
            for fn in pending_out:
                fn()

    nc.compile()
    return nc


_PROG = None


def _get_program():
    global _PROG
    if _PROG is None:
        _PROG = _build_program()
    return _PROG


def _rope_tables():
    inv_freq = (1.0 / (10000.0 ** (np.arange(0, DH, 2, dtype=np.float32) / DH)))
    invf2 = inv_freq[np.arange(128) % 32]
    ang = np.arange(S, dtype=np.float32)[None, :] * invf2[:, None].astype(np.float32)
    return (np.cos(ang).astype(np.float32), np.sin(ang).astype(np.float32))


def make_in_maps(x, W_qkv, b_qkv, W_out, b_out):
    BF = ml_dtypes.bfloat16
    x = np.asarray(x, dtype=np.float32)
    W_qkv = np.asarray(W_qkv, dtype=np.float32)
    b_qkv = np.asarray(b_qkv, dtype=np.float32)
    W_out = np.asarray(W_out, dtype=np.float32)

    xT = np.ascontiguousarray(x.reshape(S2, D).T.astype(BF))
    ct, st = _rope_tables()

    in_maps = []
    for c in range(8):
        hA, hB = 2 * c, 2 * c + 1
        cols = np.r_[hA * DH:(hA + 1) * DH, hB * DH:(hB + 1) * DH]
        Wc = np.ascontiguousarray(
            np.concatenate([W_qkv[:, off + cols] for off in (0, D, 2 * D)],
                           axis=1).astype(BF))
        bqc = np.ascontiguousarray(
            np.concatenate([b_qkv[off + cols] for off in (0, D, 2 * D)])
            .reshape(3, 128).T.astype(np.float32))
        Woc = np.ascontiguousarray(W_out[c * 128:(c + 1) * 128, :].astype(BF))
        in_maps.append(
            {"xT": xT, "W": Wc, "bqc": bqc, "Wo": Woc, "ctab": ct, "stab": st})
    return in_maps


def assemble_output(results, b_out):
    acc = results[0]["out"].astype(np.float64)
    for c in range(1, 8):
        acc += results[c]["out"]
    out = acc + np.asarray(b_out, dtype=np.float64)
    return out.reshape(B, S, D).astype(np.float32)


def kernel(x, W_qkv, b_qkv, W_out, b_out):
    nc = _get_program()
    in_maps = make_in_maps(x, W_qkv, b_qkv, W_out, b_out)
    res = run_bass_kernel_spmd(nc, in_maps, core_ids=list(range(8)))
    return assemble_output(res.results, b_out)


if __name__ == "__main__":
    rng = np.random.default_rng(0)
    ins = {
        "x": rng.standard_normal((B, S, D), dtype=np.float32),
        "W_qkv": rng.standard_normal((D, 3 * D), dtype=np.float32) / 32.0,
        "b_qkv": np.zeros(3 * D, np.float32),
        "W_out": rng.standard_normal((D, D), dtype=np.float32) / 32.0,
        "b_out": np.zeros(D, np.float32),
    }
    o = kernel(**ins)
    print("kernel ran:", o.shape, o.dtype)
